# revision 44
# baseline (speedup 1.0000x reference)
"""Trainium2 Bass kernel for nn_ConvAggregator (GNN FFT-conv aggregator).

Math: out = real(ifft2( prod_k fft2((feature @ W_aff + b_aff)[nbr_k]) )) @ W_mlp + b_mlp

Key transformation: fft2 of each 16x16 map is linear => fold the affine+FFT2
into one real matmul producing a packed 256-float spectrum per node
(Hermitian symmetry of real-input FFT: 126 conjugate-pair reps (Re,Im) +
4 self-conjugate real coefficients).  The K=16 neighbor reduction is an
elementwise complex product over packed spectra; ifft2 + W_mlp fold into a
single [256,128] matmul on the packed product.

Pipeline (all fp16; spectra scaled by alpha=2^-4 per factor, folded into
Wpack/bpack host-side, so chain intermediates stay in fp16 range; the final
activation rescales by alpha^-16 = 2^64 exactly):

  Phase 1: S = fp16(feature @ Wpack + bpack) [20096, 256] in DRAM.
    fp16 PE matmuls; bias added either by an identity-matmul accumulation
    (for chunks whose PSUM->fp16 cast runs on Act) or fused into the
    scalar_tensor_tensor cast (for chunks cast on DVE).  Feature loads and
    S writes are spread across the SP/DVE/Act DMA queues.
  Phase 2, per 512-node supertile:
    - 8x 1024-row dma_gather of neighbor spectra (rows gathered through a
      float32 view: same bytes, half the element count).
    - binary product tree over K in the free dimension: 6 wide fp16
      elementwise passes + 2 tiny self-lane passes per level, split
      DVE (2x_1p fast path) / GPSIMD.
    - posttrans: PE transposes P to spectrum-major, two matmuls with
      Wpost = Hmat @ W_mlp, bias+rescale on Act, DMA out.

Sharding: data-parallel over destination nodes, 2500 nodes/core x 8 cores;
weights + feature table replicated per core.
"""

import numpy as np

import concourse.bass as bass
import concourse.tile as tile
from concourse import bacc, mybir
from concourse.tile import add_dep_helper

F32 = mybir.dt.float32
F16 = mybir.dt.float16
I16 = mybir.dt.int16

H = 16
SW = 256            # packed spectrum width
N = 20000
NSRC = 20096        # padded to 157 full 128-row chunks
K = 16
IN_DIM = 128
OUT_DIM = 128
NCORES = 8
NPC = N // NCORES   # 2500 dest nodes per core
NPAD = 2560         # padded to 20 blocks of 128
ST = 512            # dest nodes per supertile
NB = ST // 128      # node blocks per supertile (4)
NSUP = NPAD // ST   # 5
NIDX = ST * K       # 8192 gather rows per supertile

ALPHA = 2.0 ** -4           # per-factor spectrum scale (exact in fp16)
SCALE_BACK = 2.0 ** 64      # alpha^-16 (exact in fp32)

GRP = 2048          # phase-1 feature columns per load group


# ----------------------------------------------------------------------------
# host-side constant matrices (packed FFT algebra)
# ----------------------------------------------------------------------------

def _build_rep_maps():
    seen, pairs, selfs = set(), [], []
    for u in range(H):
        for v in range(H):
            if (u, v) in seen:
                continue
            cu, cv = (-u) % H, (-v) % H
            if (cu, cv) == (u, v):
                selfs.append((u, v))
                seen.add((u, v))
            else:
                pairs.append((u, v))
                seen.add((u, v))
                seen.add((cu, cv))
    return pairs, selfs


def _build_matrices():
    """Tpack [256,256]: pre_flat -> packed ; Hmat [256,256]: packed -> h_flat.

    Packed layout: [Re(pair j) j<126 | self0 self1 | Im(pair j) j<126 | self2 self3]
    """
    pairs, selfs = _build_rep_maps()
    w = np.exp(-2j * np.pi * np.outer(np.arange(H), np.arange(H)) / H)
    F2D = np.kron(w, w)
    HID = H * H

    Tpack = np.zeros((HID, SW), dtype=np.float64)
    for j, (u, v) in enumerate(pairs):
        row = F2D[u * H + v]
        Tpack[:, j] = row.real
        Tpack[:, 128 + j] = row.imag
    for m, (u, v) in enumerate(selfs):
        col = 126 + m if m < 2 else 254 + (m - 2)
        Tpack[:, col] = F2D[u * H + v].real

    Hmat = np.zeros((SW, HID), dtype=np.float64)
    for t in range(SW):
        full = np.zeros(HID, dtype=np.complex128)
        if t < 126:
            u, v = pairs[t]
            full[u * H + v] += 1
            full[((-u) % H) * H + ((-v) % H)] += 1
        elif t < 128:
            u, v = selfs[t - 126]
            full[u * H + v] += 1
        elif t < 254:
            u, v = pairs[t - 128]
            full[u * H + v] += 1j
            full[((-u) % H) * H + ((-v) % H)] -= 1j
        else:
            u, v = selfs[2 + (t - 254)]
            full[u * H + v] += 1
        Hmat[t] = np.fft.ifft2(full.reshape(H, H)).real.flatten()
    return Tpack, Hmat


_TPACK, _HMAT = _build_matrices()


# ----------------------------------------------------------------------------
# bass module
# ----------------------------------------------------------------------------

def build_module():
    from concourse.masks import make_identity

    nc = bacc.Bacc(None, name="conv_agg", target_bir_lowering=False)
    MUL = mybir.AluOpType.mult
    ADD = mybir.AluOpType.add
    SUB = mybir.AluOpType.subtract
    COPY = mybir.ActivationFunctionType.Copy

    feat_t = nc.dram_tensor("feat_t", [IN_DIM, NSRC], F16, kind="ExternalInput")
    wpack = nc.dram_tensor("wpack", [IN_DIM, SW], F16, kind="ExternalInput")
    bpk2 = nc.dram_tensor("bpk2", [128, 2, SW], F16, kind="ExternalInput")
    wpost = nc.dram_tensor("wpost", [128, 2, OUT_DIM], F16, kind="ExternalInput")
    bmlp = nc.dram_tensor("bmlp", [OUT_DIM, 1], F32, kind="ExternalInput")
    gidx = nc.dram_tensor("gidx", [NSUP, 128, NIDX // 16], I16, kind="ExternalInput")
    out_t = nc.dram_tensor("out_t", [OUT_DIM, NPAD], F32, kind="ExternalOutput")
    s_dram = nc.dram_tensor("s_spec", [NSRC, SW], F16, kind="Internal")

    with tile.TileContext(nc) as tc:
        with tc.tile_pool(name="const", bufs=1) as cpool:
            wpack_sb = cpool.tile([IN_DIM, SW], F16)
            nc.sync.dma_start(out=wpack_sb[:], in_=wpack[:, :])
            bpk2_sb = cpool.tile([128, 2, SW], F16)
            nc.sync.dma_start(out=bpk2_sb[:], in_=bpk2[:, :, :])
            wpost_sb = cpool.tile([128, 2, OUT_DIM], F16)
            nc.sync.dma_start(out=wpost_sb[:], in_=wpost[:, :, :])
            bmlp_sb = cpool.tile([OUT_DIM, 1], F32)
            nc.sync.dma_start(out=bmlp_sb[:], in_=bmlp[:, :])
            ident = cpool.tile([128, 128], F16)
            make_identity(nc, ident[:])

            # ------------- phase 1: S = fp16(feature @ Wpack + bpack) -------
            s_writes = []
            wr_engines = [nc.sync, nc.gpsimd, nc.sync, nc.gpsimd,
                          nc.sync, nc.gpsimd, nc.sync, nc.gpsimd,
                          nc.sync, nc.gpsimd]
            with tc.tile_pool(name="p1f", bufs=3) as fpool, \
                 tc.tile_pool(name="p1s", bufs=3) as spool, \
                 tc.tile_pool(name="p1p", bufs=6, space="PSUM") as p1psum:
                g0 = 0
                gi = 0
                cast_i = 0
                while g0 < NSRC:
                    gw = min(GRP, NSRC - g0)
                    nch = gw // 128
                    ft = fpool.tile([IN_DIM, GRP], F16, tag="ft")
                    ld_eng = nc.gpsimd if gi % 2 else nc.sync
                    ld_eng.dma_start(out=ft[:, :gw], in_=feat_t[:, g0:g0 + gw])
                    stage = spool.tile([128, GRP // 128, SW], F16, tag="stage")
                    for t in range((nch + 1) // 2):
                        pw = min(2, nch - 2 * t)     # chunks in this pair
                        on_dve = (cast_i % 2 == 0)
                        cast_i += 1
                        ps = p1psum.tile([128, 2, SW], F32, tag="ps")
                        for h in range(pw):
                            c = 2 * t + h
                            nc.tensor.matmul(ps[:, h, :],
                                             lhsT=ft[:, 128 * c:128 * (c + 1)],
                                             rhs=wpack_sb[:], start=True,
                                             stop=on_dve)
                            if not on_dve:
                                # bias via identity-matmul accumulation
                                nc.tensor.matmul(ps[:, h, :], lhsT=ident[:],
                                                 rhs=bpk2_sb[:, h, :],
                                                 start=False, stop=True)
                        if on_dve:
                            # fused bias-add + fp16 cast on DVE
                            nc.vector.scalar_tensor_tensor(
                                out=stage[:, 2 * t:2 * t + pw, :],
                                in0=ps[:, :pw, :], scalar=1.0,
                                in1=bpk2_sb[:, :pw, :], op0=MUL, op1=ADD)
                        else:
                            nc.scalar.activation(stage[:, 2 * t:2 * t + pw, :],
                                                 ps[:, :pw, :], COPY)
                    dst = s_dram[g0:g0 + gw, :]
                    w = wr_engines[gi % len(wr_engines)].dma_start(
                        out=dst.rearrange("(c p) e -> p c e", p=128),
                        in_=stage[:, :nch, :])
                    s_writes.append(w)
                    gi += 1
                    g0 += gw

            join = nc.sync.nop(nofuse=True, hint="phase1_done")
            for w in s_writes:
                add_dep_helper(join.ins, w.ins, reason="gather waits on S table")

            # ------------- phase 2: gather + product tree + posttrans -------
            s32 = s_dram[:, :].bitcast(F32)              # [NSRC, 128] f32 view
            with tc.tile_pool(name="p2i", bufs=2) as ipool, \
                 tc.tile_pool(name="p2m", bufs=2) as mpool, \
                 tc.tile_pool(name="p2p", bufs=2) as ppool, \
                 tc.tile_pool(name="p2p0", bufs=1) as p1pool, \
                 tc.tile_pool(name="p2t", bufs=1) as tpool, \
                 tc.tile_pool(name="p2x", bufs=2) as xpool, \
                 tc.tile_pool(name="p2o", bufs=2) as opool, \
                 tc.tile_pool(name="p2ps", bufs=2, space="PSUM") as p2psum:
                for s in range(NSUP):
                    ix = ipool.tile([128, NIDX // 16], I16, tag="ix",
                                    name=f"ix{s}")
                    nc.sync.dma_start(out=ix[:], in_=gidx[s, :, :])
                    mb = mpool.tile([128, 2 * 32, SW], F16, tag="mb",
                                    name=f"mb{s}")
                    for q in range(NIDX // 1024):
                        g = nc.gpsimd.dma_gather(
                            mb[:, 8 * q:8 * (q + 1), :].bitcast(F32),
                            s32, ix[:, 64 * q:64 * (q + 1)],
                            1024, 1024, SW // 2, elem_step=SW // 2)
                        add_dep_helper(g.ins, join.ins,
                                       reason="gather after S ready")

                    # binary product tree over K (mailbox col = k*NB + b)
                    cur = mb
                    for lvl, w in enumerate((32, 16, 8, 4)):
                        A = cur[:, 0:w, :]
                        B = cur[:, w:2 * w, :]
                        if lvl == 0:
                            pn = p1pool.tile([128, w, SW], F16, tag="p0")
                        else:
                            pn = ppool.tile([128, w, SW], F16, tag=f"p{lvl}")
                        t0 = tpool.tile([128, 32, 126], F16, tag="t0")
                        t1 = tpool.tile([128, 32, 126], F16, tag="t1")
                        t2 = tpool.tile([128, 32, 126], F16, tag="t2")
                        t3 = tpool.tile([128, 32, 126], F16, tag="t3")
                        aR, aI = A[:, :, 0:126], A[:, :, 128:254]
                        bR, bI = B[:, :, 0:126], B[:, :, 128:254]
                        # products first (cross-engine overlap), then combines
                        nc.vector.tensor_tensor(t0[:, :w, :], aR, bR, op=MUL)
                        nc.vector.tensor_tensor(t2[:, :w, :], aR, bI, op=MUL)
                        nc.gpsimd.tensor_tensor(t1[:, :w, :], aI, bI, op=MUL)
                        if lvl < 2:
                            nc.gpsimd.tensor_tensor(t3[:, :w, :], aI, bR, op=MUL)
                        else:
                            nc.vector.tensor_tensor(t3[:, :w, :], aI, bR, op=MUL)
                        nc.vector.tensor_tensor(pn[:, :, 0:126], t0[:, :w, :],
                                                t1[:, :w, :], op=SUB)
                        nc.vector.tensor_tensor(pn[:, :, 128:254], t2[:, :w, :],
                                                t3[:, :w, :], op=ADD)
                        # self-conjugate (plain real product) lanes
                        nc.gpsimd.tensor_tensor(pn[:, :, 126:128],
                                                A[:, :, 126:128],
                                                B[:, :, 126:128], op=MUL)
                        nc.gpsimd.tensor_tensor(pn[:, :, 254:256],
                                                A[:, :, 254:256],
                                                B[:, :, 254:256], op=MUL)
                        cur = pn

                    # posttrans: transpose P to spec-major, matmul, bias+scale
                    pt0 = xpool.tile([128, ST], F16, tag="pt0")
                    pt1 = xpool.tile([128, ST], F16, tag="pt1")
                    ptT = [pt0, pt1]
                    for h in range(2):
                        tp = p2psum.tile([128, NB, 128], F16, tag=f"tp{h}")
                        for b in range(NB):
                            nc.tensor.transpose(
                                tp[:, b, :], cur[:, b, 128 * h:128 * (h + 1)],
                                identity=ident[:])
                        nc.scalar.activation(ptT[h][:], tp[:], COPY)
                    ops = p2psum.tile([128, ST], F32, tag="ops")
                    nc.tensor.matmul(ops[:], lhsT=wpost_sb[:, 0, :],
                                     rhs=pt0[:], start=True, stop=False)
                    nc.tensor.matmul(ops[:], lhsT=wpost_sb[:, 1, :],
                                     rhs=pt1[:], start=False, stop=True)
                    ob = opool.tile([OUT_DIM, ST], F32, tag="ob")
                    nc.scalar.activation(ob[:], ops[:],
                                         mybir.ActivationFunctionType.Identity,
                                         bias=bmlp_sb[:, 0:1], scale=SCALE_BACK)
                    nc.sync.dma_start(out=out_t[:, s * ST:(s + 1) * ST],
                                      in_=ob[:])

    nc.compile()
    return nc


# ----------------------------------------------------------------------------
# host wrapper
# ----------------------------------------------------------------------------

_NC_CACHE = None


def _get_module():
    global _NC_CACHE
    if _NC_CACHE is None:
        _NC_CACHE = build_module()
    return _NC_CACHE


def _make_gidx(neighbors):
    """Per-core gather index tensors [NCORES, NSUP, 128, NIDX//16] int16.

    Flat gather order i = (k*NB + b)*128 + p so mailbox col = k*NB + b
    (k-major).  Indices are wrapped into 16 rows (idx[q, j] = flat[j*16+q])
    and replicated across the 128 partitions.
    """
    nb = np.asarray(neighbors).astype(np.int64)
    out = np.zeros((NCORES, NSUP, 128, NIDX // 16), dtype=np.int16)
    for c in range(NCORES):
        nbp = np.zeros((NPAD, K), np.int64)
        nbp[:NPC] = nb[c * NPC:(c + 1) * NPC]
        for s in range(NSUP):
            blk = nbp[s * ST:(s + 1) * ST]                    # [ST, K]
            t = blk.reshape(NB, 128, K)                       # [b, p, k]
            flat = np.transpose(t, (2, 0, 1)).reshape(NIDX)   # i=(k*NB+b)*128+p
            wrapped = flat.reshape(NIDX // 16, 16).T          # [16, NIDX//16]
            out[c, s] = np.tile(wrapped, (8, 1)).astype(np.int16)
    return out


def _make_inputs(feature, neighbors, W_aff, b_aff, W_mlp, b_mlp):
    feature = np.asarray(feature, np.float32)
    Wpack = ((np.asarray(W_aff, np.float64) @ _TPACK) * ALPHA).astype(np.float16)
    bpack = ((np.asarray(b_aff, np.float64) @ _TPACK) * ALPHA).astype(np.float16)
    Wpost = (_HMAT @ np.asarray(W_mlp, np.float64)).astype(np.float16)

    feat_pad = np.zeros((NSRC, IN_DIM), np.float16)
    feat_pad[:N] = feature.astype(np.float16)
    feat_tr = np.ascontiguousarray(feat_pad.T)                     # [128, NSRC]
    bpk2_rep = np.ascontiguousarray(
        np.broadcast_to(bpack[None, None, :], (128, 2, SW)).copy())
    wpost_h = np.ascontiguousarray(
        np.stack([Wpost[0:128, :], Wpost[128:256, :]], axis=1))    # [128,2,128]
    bmlp_col = np.ascontiguousarray(
        np.asarray(b_mlp, np.float32).reshape(OUT_DIM, 1))
    gidx = _make_gidx(neighbors)

    in_maps = []
    for c in range(NCORES):
        in_maps.append({
            "feat_t": feat_tr,
            "wpack": np.ascontiguousarray(Wpack),
            "bpk2": bpk2_rep,
            "wpost": wpost_h,
            "bmlp": bmlp_col,
            "gidx": np.ascontiguousarray(gidx[c]),
        })
    return in_maps


def kernel(feature, neighbors, W_aff, b_aff, W_mlp, b_mlp):
    from concourse import bass_utils

    nc = _get_module()
    in_maps = _make_inputs(feature, neighbors, W_aff, b_aff, W_mlp, b_mlp)
    res = bass_utils.run_bass_kernel_spmd(nc, in_maps, core_ids=list(range(NCORES)))
    out = np.empty((N, OUT_DIM), dtype=np.float32)
    for c in range(NCORES):
        out[c * NPC:(c + 1) * NPC] = res.results[c]["out_t"][:, :NPC].T
    return out


# revision 45
# speedup vs baseline: 1.0007x; 1.0007x over previous
"""Trainium2 Bass kernel for nn_ConvAggregator (GNN FFT-conv aggregator).

Math: out = real(ifft2( prod_k fft2((feature @ W_aff + b_aff)[nbr_k]) )) @ W_mlp + b_mlp

Key transformation: fft2 of each 16x16 map is linear => fold the affine+FFT2
into one real matmul producing a packed 256-float spectrum per node
(Hermitian symmetry of real-input FFT: 126 conjugate-pair reps (Re,Im) +
4 self-conjugate real coefficients).  The K=16 neighbor reduction is an
elementwise complex product over packed spectra; ifft2 + W_mlp fold into a
single [256,128] matmul on the packed product.

Pipeline (all fp16; spectra scaled by alpha=2^-4 per factor, folded into
Wpack/bpack host-side, so chain intermediates stay in fp16 range; the final
activation rescales by alpha^-16 = 2^64 exactly):

  Phase 1: S = fp16(feature @ Wpack + bpack) [20096, 256] in DRAM.
    fp16 PE matmuls; bias added either by an identity-matmul accumulation
    (for chunks whose PSUM->fp16 cast runs on Act) or fused into the
    scalar_tensor_tensor cast (for chunks cast on DVE).  Feature loads and
    S writes are spread across the SP/DVE/Act DMA queues.
  Phase 2, per 512-node supertile:
    - 8x 1024-row dma_gather of neighbor spectra (rows gathered through a
      float32 view: same bytes, half the element count).
    - binary product tree over K in the free dimension: 6 wide fp16
      elementwise passes + 2 tiny self-lane passes per level, split
      DVE (2x_1p fast path) / GPSIMD.
    - posttrans: PE transposes P to spectrum-major, two matmuls with
      Wpost = Hmat @ W_mlp, bias+rescale on Act, DMA out.

Sharding: data-parallel over destination nodes, 2500 nodes/core x 8 cores;
weights + feature table replicated per core.
"""

import numpy as np

import concourse.bass as bass
import concourse.tile as tile
from concourse import bacc, mybir
from concourse.tile import add_dep_helper

F32 = mybir.dt.float32
F16 = mybir.dt.float16
I16 = mybir.dt.int16

H = 16
SW = 256            # packed spectrum width
N = 20000
NSRC = 20096        # padded to 157 full 128-row chunks
K = 16
IN_DIM = 128
OUT_DIM = 128
NCORES = 8
NPC = N // NCORES   # 2500 dest nodes per core
NPAD = 2560         # padded to 20 blocks of 128
ST = 512            # dest nodes per supertile
NB = ST // 128      # node blocks per supertile (4)
NSUP = NPAD // ST   # 5
NIDX = ST * K       # 8192 gather rows per supertile

ALPHA = 2.0 ** -4           # per-factor spectrum scale (exact in fp16)
SCALE_BACK = 2.0 ** 64      # alpha^-16 (exact in fp32)

GRP = 2048          # phase-1 feature columns per load group


# ----------------------------------------------------------------------------
# host-side constant matrices (packed FFT algebra)
# ----------------------------------------------------------------------------

def _build_rep_maps():
    seen, pairs, selfs = set(), [], []
    for u in range(H):
        for v in range(H):
            if (u, v) in seen:
                continue
            cu, cv = (-u) % H, (-v) % H
            if (cu, cv) == (u, v):
                selfs.append((u, v))
                seen.add((u, v))
            else:
                pairs.append((u, v))
                seen.add((u, v))
                seen.add((cu, cv))
    return pairs, selfs


def _build_matrices():
    """Tpack [256,256]: pre_flat -> packed ; Hmat [256,256]: packed -> h_flat.

    Packed layout: [Re(pair j) j<126 | self0 self1 | Im(pair j) j<126 | self2 self3]
    """
    pairs, selfs = _build_rep_maps()
    w = np.exp(-2j * np.pi * np.outer(np.arange(H), np.arange(H)) / H)
    F2D = np.kron(w, w)
    HID = H * H

    Tpack = np.zeros((HID, SW), dtype=np.float64)
    for j, (u, v) in enumerate(pairs):
        row = F2D[u * H + v]
        Tpack[:, j] = row.real
        Tpack[:, 128 + j] = row.imag
    for m, (u, v) in enumerate(selfs):
        col = 126 + m if m < 2 else 254 + (m - 2)
        Tpack[:, col] = F2D[u * H + v].real

    Hmat = np.zeros((SW, HID), dtype=np.float64)
    for t in range(SW):
        full = np.zeros(HID, dtype=np.complex128)
        if t < 126:
            u, v = pairs[t]
            full[u * H + v] += 1
            full[((-u) % H) * H + ((-v) % H)] += 1
        elif t < 128:
            u, v = selfs[t - 126]
            full[u * H + v] += 1
        elif t < 254:
            u, v = pairs[t - 128]
            full[u * H + v] += 1j
            full[((-u) % H) * H + ((-v) % H)] -= 1j
        else:
            u, v = selfs[2 + (t - 254)]
            full[u * H + v] += 1
        Hmat[t] = np.fft.ifft2(full.reshape(H, H)).real.flatten()
    return Tpack, Hmat


_TPACK, _HMAT = _build_matrices()


# ----------------------------------------------------------------------------
# bass module
# ----------------------------------------------------------------------------

def build_module():
    from concourse.masks import make_identity

    nc = bacc.Bacc(None, name="conv_agg", target_bir_lowering=False)
    MUL = mybir.AluOpType.mult
    ADD = mybir.AluOpType.add
    SUB = mybir.AluOpType.subtract
    COPY = mybir.ActivationFunctionType.Copy

    feat_t = nc.dram_tensor("feat_t", [IN_DIM, NSRC], F16, kind="ExternalInput")
    wpack = nc.dram_tensor("wpack", [IN_DIM, SW], F16, kind="ExternalInput")
    bpk2 = nc.dram_tensor("bpk2", [128, 2, SW], F16, kind="ExternalInput")
    wpost = nc.dram_tensor("wpost", [128, 2, OUT_DIM], F16, kind="ExternalInput")
    bmlp = nc.dram_tensor("bmlp", [OUT_DIM, 1], F32, kind="ExternalInput")
    gidx = nc.dram_tensor("gidx", [NSUP, 128, NIDX // 16], I16, kind="ExternalInput")
    out_t = nc.dram_tensor("out_t", [OUT_DIM, NPAD], F32, kind="ExternalOutput")
    s_dram = nc.dram_tensor("s_spec", [NSRC, SW], F16, kind="Internal")

    with tile.TileContext(nc) as tc:
        with tc.tile_pool(name="const", bufs=1) as cpool:
            wpack_sb = cpool.tile([IN_DIM, SW], F16)
            nc.sync.dma_start(out=wpack_sb[:], in_=wpack[:, :])
            bpk2_sb = cpool.tile([128, 2, SW], F16)
            nc.sync.dma_start(out=bpk2_sb[:], in_=bpk2[:, :, :])
            wpost_sb = cpool.tile([128, 2, OUT_DIM], F16)
            nc.sync.dma_start(out=wpost_sb[:], in_=wpost[:, :, :])
            bmlp_sb = cpool.tile([OUT_DIM, 1], F32)
            nc.sync.dma_start(out=bmlp_sb[:], in_=bmlp[:, :])
            ident = cpool.tile([128, 128], F16)
            make_identity(nc, ident[:])

            # ------------- phase 1: S = fp16(feature @ Wpack + bpack) -------
            s_writes = []
            wr_engines = [nc.sync, nc.gpsimd, nc.sync, nc.gpsimd,
                          nc.sync, nc.gpsimd, nc.sync, nc.gpsimd,
                          nc.sync, nc.gpsimd]
            with tc.tile_pool(name="p1f", bufs=3) as fpool, \
                 tc.tile_pool(name="p1s", bufs=3) as spool, \
                 tc.tile_pool(name="p1p", bufs=6, space="PSUM") as p1psum:
                g0 = 0
                gi = 0
                cast_i = 0
                while g0 < NSRC:
                    gw = min(GRP, NSRC - g0)
                    nch = gw // 128
                    ft = fpool.tile([IN_DIM, GRP], F16, tag="ft")
                    ld_eng = nc.gpsimd if gi % 2 else nc.sync
                    ld_eng.dma_start(out=ft[:, :gw], in_=feat_t[:, g0:g0 + gw])
                    stage = spool.tile([128, GRP // 128, SW], F16, tag="stage")
                    for t in range((nch + 1) // 2):
                        pw = min(2, nch - 2 * t)     # chunks in this pair
                        on_dve = (cast_i % 2 == 0)
                        cast_i += 1
                        ps = p1psum.tile([128, 2, SW], F32, tag="ps")
                        for h in range(pw):
                            c = 2 * t + h
                            nc.tensor.matmul(ps[:, h, :],
                                             lhsT=ft[:, 128 * c:128 * (c + 1)],
                                             rhs=wpack_sb[:], start=True,
                                             stop=on_dve)
                            if not on_dve:
                                # bias via identity-matmul accumulation
                                nc.tensor.matmul(ps[:, h, :], lhsT=ident[:],
                                                 rhs=bpk2_sb[:, h, :],
                                                 start=False, stop=True)
                        if on_dve:
                            # fused bias-add + fp16 cast on DVE
                            nc.vector.scalar_tensor_tensor(
                                out=stage[:, 2 * t:2 * t + pw, :],
                                in0=ps[:, :pw, :], scalar=1.0,
                                in1=bpk2_sb[:, :pw, :], op0=MUL, op1=ADD)
                        else:
                            nc.scalar.activation(stage[:, 2 * t:2 * t + pw, :],
                                                 ps[:, :pw, :], COPY)
                    dst = s_dram[g0:g0 + gw, :]
                    w = wr_engines[gi % len(wr_engines)].dma_start(
                        out=dst.rearrange("(c p) e -> p c e", p=128),
                        in_=stage[:, :nch, :])
                    s_writes.append(w)
                    gi += 1
                    g0 += gw

            join = nc.sync.nop(nofuse=True, hint="phase1_done")
            for w in s_writes:
                add_dep_helper(join.ins, w.ins, reason="gather waits on S table")

            # ------------- phase 2: gather + product tree + posttrans -------
            s32 = s_dram[:, :].bitcast(F32)              # [NSRC, 128] f32 view
            with tc.tile_pool(name="p2i", bufs=2) as ipool, \
                 tc.tile_pool(name="p2m", bufs=2) as mpool, \
                 tc.tile_pool(name="p2p", bufs=2) as ppool, \
                 tc.tile_pool(name="p2p0", bufs=1) as p1pool, \
                 tc.tile_pool(name="p2t", bufs=1) as tpool, \
                 tc.tile_pool(name="p2x", bufs=2) as xpool, \
                 tc.tile_pool(name="p2o", bufs=2) as opool, \
                 tc.tile_pool(name="p2ps", bufs=2, space="PSUM") as p2psum:
                for s in range(NSUP):
                    ix = ipool.tile([128, NIDX // 16], I16, tag="ix",
                                    name=f"ix{s}")
                    nc.sync.dma_start(out=ix[:], in_=gidx[s, :, :])
                    mb = mpool.tile([128, 2 * 32, SW], F16, tag="mb",
                                    name=f"mb{s}")
                    for q in range(NIDX // 1024):
                        g = nc.gpsimd.dma_gather(
                            mb[:, 8 * q:8 * (q + 1), :].bitcast(F32),
                            s32, ix[:, 64 * q:64 * (q + 1)],
                            1024, 1024, SW // 2, elem_step=SW // 2)
                        add_dep_helper(g.ins, join.ins,
                                       reason="gather after S ready")

                    # binary product tree over K (mailbox col = k*NB + b)
                    cur = mb
                    for lvl, w in enumerate((32, 16, 8, 4)):
                        A = cur[:, 0:w, :]
                        B = cur[:, w:2 * w, :]
                        if lvl == 0:
                            pn = p1pool.tile([128, w, SW], F16, tag="p0")
                        else:
                            pn = ppool.tile([128, w, SW], F16, tag=f"p{lvl}")
                        t0 = tpool.tile([128, 32, 126], F16, tag="t0")
                        t1 = tpool.tile([128, 32, 126], F16, tag="t1")
                        t2 = tpool.tile([128, 32, 126], F16, tag="t2")
                        t3 = tpool.tile([128, 32, 126], F16, tag="t3")
                        aR, aI = A[:, :, 0:126], A[:, :, 128:254]
                        bR, bI = B[:, :, 0:126], B[:, :, 128:254]
                        # products first (cross-engine overlap), then combines
                        nc.vector.tensor_tensor(t0[:, :w, :], aR, bR, op=MUL)
                        nc.vector.tensor_tensor(t2[:, :w, :], aR, bI, op=MUL)
                        nc.gpsimd.tensor_tensor(t1[:, :w, :], aI, bI, op=MUL)
                        if lvl < 2:
                            nc.gpsimd.tensor_tensor(t3[:, :w, :], aI, bR, op=MUL)
                        else:
                            nc.vector.tensor_tensor(t3[:, :w, :], aI, bR, op=MUL)
                        nc.vector.tensor_tensor(pn[:, :, 0:126], t0[:, :w, :],
                                                t1[:, :w, :], op=SUB)
                        nc.vector.tensor_tensor(pn[:, :, 128:254], t2[:, :w, :],
                                                t3[:, :w, :], op=ADD)
                        # self-conjugate (plain real product) lanes (on DVE:
                        # tiny, and keeps Pool's queue clear for next t1/t3)
                        nc.vector.tensor_tensor(pn[:, :, 126:128],
                                                A[:, :, 126:128],
                                                B[:, :, 126:128], op=MUL)
                        nc.vector.tensor_tensor(pn[:, :, 254:256],
                                                A[:, :, 254:256],
                                                B[:, :, 254:256], op=MUL)
                        cur = pn

                    # posttrans: transpose P to spec-major, matmul, bias+scale
                    pt0 = xpool.tile([128, ST], F16, tag="pt0")
                    pt1 = xpool.tile([128, ST], F16, tag="pt1")
                    ptT = [pt0, pt1]
                    for h in range(2):
                        tp = p2psum.tile([128, NB, 128], F16, tag=f"tp{h}")
                        for b in range(NB):
                            nc.tensor.transpose(
                                tp[:, b, :], cur[:, b, 128 * h:128 * (h + 1)],
                                identity=ident[:])
                        nc.scalar.activation(ptT[h][:], tp[:], COPY)
                    ops = p2psum.tile([128, ST], F32, tag="ops")
                    nc.tensor.matmul(ops[:], lhsT=wpost_sb[:, 0, :],
                                     rhs=pt0[:], start=True, stop=False)
                    nc.tensor.matmul(ops[:], lhsT=wpost_sb[:, 1, :],
                                     rhs=pt1[:], start=False, stop=True)
                    ob = opool.tile([OUT_DIM, ST], F32, tag="ob")
                    nc.scalar.activation(ob[:], ops[:],
                                         mybir.ActivationFunctionType.Identity,
                                         bias=bmlp_sb[:, 0:1], scale=SCALE_BACK)
                    nc.sync.dma_start(out=out_t[:, s * ST:(s + 1) * ST],
                                      in_=ob[:])

    nc.compile()
    return nc


# ----------------------------------------------------------------------------
# host wrapper
# ----------------------------------------------------------------------------

_NC_CACHE = None


def _get_module():
    global _NC_CACHE
    if _NC_CACHE is None:
        _NC_CACHE = build_module()
    return _NC_CACHE


def _make_gidx(neighbors):
    """Per-core gather index tensors [NCORES, NSUP, 128, NIDX//16] int16.

    Flat gather order i = (k*NB + b)*128 + p so mailbox col = k*NB + b
    (k-major).  Indices are wrapped into 16 rows (idx[q, j] = flat[j*16+q])
    and replicated across the 128 partitions.
    """
    nb = np.asarray(neighbors).astype(np.int64)
    out = np.zeros((NCORES, NSUP, 128, NIDX // 16), dtype=np.int16)
    for c in range(NCORES):
        nbp = np.zeros((NPAD, K), np.int64)
        nbp[:NPC] = nb[c * NPC:(c + 1) * NPC]
        for s in range(NSUP):
            blk = nbp[s * ST:(s + 1) * ST]                    # [ST, K]
            t = blk.reshape(NB, 128, K)                       # [b, p, k]
            flat = np.transpose(t, (2, 0, 1)).reshape(NIDX)   # i=(k*NB+b)*128+p
            wrapped = flat.reshape(NIDX // 16, 16).T          # [16, NIDX//16]
            out[c, s] = np.tile(wrapped, (8, 1)).astype(np.int16)
    return out


def _make_inputs(feature, neighbors, W_aff, b_aff, W_mlp, b_mlp):
    feature = np.asarray(feature, np.float32)
    Wpack = ((np.asarray(W_aff, np.float64) @ _TPACK) * ALPHA).astype(np.float16)
    bpack = ((np.asarray(b_aff, np.float64) @ _TPACK) * ALPHA).astype(np.float16)
    Wpost = (_HMAT @ np.asarray(W_mlp, np.float64)).astype(np.float16)

    feat_pad = np.zeros((NSRC, IN_DIM), np.float16)
    feat_pad[:N] = feature.astype(np.float16)
    feat_tr = np.ascontiguousarray(feat_pad.T)                     # [128, NSRC]
    bpk2_rep = np.ascontiguousarray(
        np.broadcast_to(bpack[None, None, :], (128, 2, SW)).copy())
    wpost_h = np.ascontiguousarray(
        np.stack([Wpost[0:128, :], Wpost[128:256, :]], axis=1))    # [128,2,128]
    bmlp_col = np.ascontiguousarray(
        np.asarray(b_mlp, np.float32).reshape(OUT_DIM, 1))
    gidx = _make_gidx(neighbors)

    in_maps = []
    for c in range(NCORES):
        in_maps.append({
            "feat_t": feat_tr,
            "wpack": np.ascontiguousarray(Wpack),
            "bpk2": bpk2_rep,
            "wpost": wpost_h,
            "bmlp": bmlp_col,
            "gidx": np.ascontiguousarray(gidx[c]),
        })
    return in_maps


def kernel(feature, neighbors, W_aff, b_aff, W_mlp, b_mlp):
    from concourse import bass_utils

    nc = _get_module()
    in_maps = _make_inputs(feature, neighbors, W_aff, b_aff, W_mlp, b_mlp)
    res = bass_utils.run_bass_kernel_spmd(nc, in_maps, core_ids=list(range(NCORES)))
    out = np.empty((N, OUT_DIM), dtype=np.float32)
    for c in range(NCORES):
        out[c * NPC:(c + 1) * NPC] = res.results[c]["out_t"][:, :NPC].T
    return out
